# revision 1
# baseline (speedup 1.0000x reference)
"""Bahdanau-attention GRU cell fused Trainium2 kernel.

Sharding: data-parallel over batch across 8 NeuronCores (4 batch rows per
core, weights replicated, no collectives).

Math per core (b=4 local batch rows, T=2048, F=U=512):
  pre^T[u,t]  = Ua^T ann^T + (Wa^T h + Wa_bias + Ua_bias)      (PE, bf16)
  (optional fp8 DoubleRow path exists behind build(fp8=True); bf16 measured
  faster end-to-end on hardware)
  tanh fused on ScalarE with per-partition bias
  scores      = Va . tanh(pre)      (PE matmul, Va replicated across M=128)
  p = exp(scores) (no max-sub; |scores| <= sum|Va| ~ 20, safe in fp32),
  Z via activation accum_out
  c_unnorm^T[f] = sum_t ann^T[f,t] * p[t]  (DVE mult + ScalarE accum pass)
  GRU gates: one PSUM accumulation of x@K + c@AK + h@RK[:,:2U] + biases,
  hard-sigmoid/tanh epilogue, h_new = z*h + (1-z)*hh
"""

import sys

if "/opt/trn_rl_repo" not in sys.path:
    sys.path.insert(0, "/opt/trn_rl_repo")

import numpy as np

import concourse.bass as bass
import concourse.tile as tile
from concourse import bacc, bass_utils, mybir
from concourse.masks import make_identity

F32 = mybir.dt.float32
BF16 = mybir.dt.bfloat16
FP8 = mybir.dt.float8e4
AF = mybir.ActivationFunctionType
ALU = mybir.AluOpType

B, T, F, U = 32, 2048, 512, 512
NCORES = 8
BL = B // NCORES          # 4 local batch rows
TT = 512                  # T-tile (free dim of matmuls)
NTT = T // TT             # 4
NS = TT // 128            # 4 t-subtiles per T-tile
NFB = F // 128            # 4 f blocks
NUB = U // 128            # 4 u blocks
U3 = 3 * U


def build(reps=1, fp8=False):
    nc = bacc.Bacc("TRN2", target_bir_lowering=False, debug=False)

    def din(name, shape):
        return nc.dram_tensor(name, shape, F32, kind="ExternalInput").ap()

    d_x = din("x", [BL, F])
    d_h = din("h", [BL, U])
    d_ann = din("annotations", [BL, T, F])
    d_k = din("kernel", [F, U3])
    d_rk = din("recurrent_kernel", [U, U3])
    d_ak = din("attention_kernel", [F, U3])
    d_wa = din("Wa", [U, U])
    d_ua = din("Ua", [F, U])
    d_va = din("Va", [U])
    d_bias = din("bias", [U3])
    d_abias = din("attention_bias", [U3])
    d_wab = din("Wa_bias", [U])
    d_uab = din("Ua_bias", [U])
    d_out = nc.dram_tensor("h_new", [BL, U], F32, kind="ExternalOutput").ap()

    with tile.TileContext(nc) as tc:
        with (
            tc.tile_pool(name="const", bufs=1) as const,
            tc.tile_pool(name="state", bufs=2) as state,
            tc.tile_pool(name="annio", bufs=3) as annio,
            tc.tile_pool(name="annT_p", bufs=3) as annT_p,
            tc.tile_pool(name="tanh_p", bufs=2) as tanh_p,
            tc.tile_pool(name="pbc_p", bufs=2) as pbc_p,
            tc.tile_pool(name="scr_p", bufs=2) as scr_p,
            tc.tile_pool(name="ps_tp", bufs=3, space="PSUM") as ps_tp,
            tc.tile_pool(name="ps_pre", bufs=3, space="PSUM") as ps_pre,
            tc.tile_pool(name="ps_sc", bufs=2, space="PSUM") as ps_sc,
        ):
            # ---------------- constants / weights ----------------
            ident = const.tile([128, 128], BF16)
            make_identity(nc, ident[:])

            ones4 = const.tile([1, BL], BF16)
            nc.vector.memset(ones4[:], 1.0)
            ident_f1 = const.tile([1, 1], F32)
            nc.vector.memset(ident_f1[:], 1.0)

            # first annotation tile: issue before any weight DMA so the PE
            # pipeline can start transposing as early as possible
            ann_r0 = d_ann.rearrange("b (tt s p) f -> b tt s p f", p=128, s=NS)
            a_nat0 = annio.tile([128, NS, F], BF16, tag="ann_nat", name="a_nat_first")
            nc.gpsimd.dma_start(
                out=a_nat0[:], in_=ann_r0[0, 0].rearrange("s p f -> p s f")
            )

            ua_sb = const.tile([128, NFB, U], BF16)
            nc.gpsimd.dma_start(
                out=ua_sb[:], in_=d_ua.rearrange("(fb p) u -> p fb u", p=128)
            )
            wa_sb = const.tile([128, NUB, U], BF16)
            nc.gpsimd.dma_start(
                out=wa_sb[:], in_=d_wa.rearrange("(jb p) u -> p jb u", p=128)
            )
            if fp8:
                ua8 = const.tile([128, NFB, U], FP8)
                nc.vector.tensor_copy(ua8[:], ua_sb[:])

            # small vectors: fast HWDGE fp32 loads + on-chip casts (the SWDGE
            # queue is busy with the big annotation/weight streams)
            def row_load(dram_ap, width, nm):
                t32 = const.tile([1, width], F32, name=nm + "_f32")
                nc.sync.dma_start(out=t32[:], in_=dram_ap)
                t16 = const.tile([1, width], BF16, name=nm)
                nc.vector.tensor_copy(t16[:], t32[:])
                return t16

            va_row = row_load(d_va.rearrange("(a u) -> a u", a=1), U, "va_row")
            wab_row = row_load(d_wab.rearrange("(a u) -> a u", a=1), U, "wab_row")
            uab_row = row_load(d_uab.rearrange("(a u) -> a u", a=1), U, "uab_row")
            bias_row = row_load(d_bias.rearrange("(a u) -> a u", a=1), U3, "bias_row")
            abias_row = row_load(d_abias.rearrange("(a u) -> a u", a=1), U3, "abias_row")

            x_f32 = const.tile([BL, F], F32)
            nc.sync.dma_start(out=x_f32[:], in_=d_x)
            x_bf = const.tile([BL, F], BF16)
            nc.vector.tensor_copy(x_bf[:], x_f32[:])
            h_f32 = const.tile([BL, U], F32)
            nc.sync.dma_start(out=h_f32[:], in_=d_h)
            h_bf = const.tile([BL, U], BF16)
            nc.vector.tensor_copy(h_bf[:], h_f32[:])

            # GRU weights: tiles allocated once; the chunked loads are
            # interleaved into the main loop (once per rep).
            k_sb = const.tile([128, NFB, U3], BF16)
            rk_sb = const.tile([128, NUB, U3], BF16)
            ak_sb = const.tile([128, NFB, U3], BF16)
            k_r = d_k.rearrange("(fb p) u -> p fb u", p=128)
            rk_r = d_rk.rearrange("(fb p) u -> p fb u", p=128)
            ak_r = d_ak.rearrange("(fb p) u -> p fb u", p=128)
            gru_w_chunks = []
            for fb in range(NFB):
                gru_w_chunks.append((k_sb, k_r, fb))
                gru_w_chunks.append((rk_sb, rk_r, fb))
                gru_w_chunks.append((ak_sb, ak_r, fb))

            # VaT replicated: va_rep[p, ub, j] = Va[ub*128+p] for all j
            va_rep = const.tile([128, NUB, 128], BF16)
            for ub in range(NUB):
                tp = ps_tp.tile([128, 128], BF16, tag="tp")
                nc.tensor.transpose(
                    tp[:, 0:1], va_row[0:1, 128 * ub : 128 * (ub + 1)], ident[0:1, 0:1]
                )
                nc.vector.tensor_copy(
                    va_rep[:, ub, :], tp[:, 0:1].to_broadcast([128, 128])
                )

            # x^T, h^T  (transpose [4,128] chunks -> [128,4])
            xT = const.tile([128, NFB, BL], BF16)
            hT = const.tile([128, NUB, BL], BF16)
            for jb in range(NFB):
                tp = ps_tp.tile([128, 128], BF16, tag="tp")
                nc.tensor.transpose(
                    tp[:, 0:BL], x_bf[0:BL, 128 * jb : 128 * (jb + 1)], ident[0:BL, 0:BL]
                )
                nc.any.tensor_copy(xT[:, jb, :], tp[:, 0:BL])
            for jb in range(NUB):
                tp = ps_tp.tile([128, 128], BF16, tag="tp")
                nc.tensor.transpose(
                    tp[:, 0:BL], h_bf[0:BL, 128 * jb : 128 * (jb + 1)], ident[0:BL, 0:BL]
                )
                nc.any.tensor_copy(hT[:, jb, :], tp[:, 0:BL])

            # q^T[u, b] = Wa^T h^T + Wa_bias + Ua_bias
            # (emitted inside the first loop iteration, after its transposes,
            # so the PE does not stall on the Wa load before it can start on
            # the first annotation tile)
            qT = const.tile([128, NUB, BL], F32)

            def emit_qT():
                for ub in range(NUB):
                    qp = ps_sc.tile([128, TT], F32, tag="sc", name=f"qp{ub}")
                    for jb in range(NUB):
                        nc.tensor.matmul(
                            qp[:, 0:BL],
                            wa_sb[:, jb, 128 * ub : 128 * (ub + 1)],
                            hT[:, jb, :],
                            start=(jb == 0),
                            stop=False,
                        )
                    nc.tensor.matmul(
                        qp[:, 0:BL],
                        wab_row[0:1, 128 * ub : 128 * (ub + 1)],
                        ones4[:],
                        start=False,
                        stop=False,
                    )
                    nc.tensor.matmul(
                        qp[:, 0:BL],
                        uab_row[0:1, 128 * ub : 128 * (ub + 1)],
                        ones4[:],
                        start=False,
                        stop=True,
                    )
                    nc.any.tensor_copy(qT[:, ub, :], qp[:, 0:BL])

            ann_r = d_ann.rearrange("b (tt s p) f -> b tt s p f", p=128, s=NS)

            for _rep in range(reps):
                # per-rep accumulators
                ztile = state.tile([128, BL * NTT], F32, name=f"ztile{_rep}", tag="ztile")
                cpart = state.tile([128, NFB, NTT, BL], F32, name=f"cpart{_rep}",
                                   tag="cpart")

                # ---------------- main attention loop ----------------
                for b in range(BL):
                    for tt in range(NTT):
                        it = b * NTT + tt
                        # stream in the natural-layout tile, cast fp32->bf16
                        if _rep == 0 and it == 0:
                            a_nat = a_nat0
                        else:
                            a_nat = annio.tile([128, NS, F], BF16, tag="ann_nat",
                                               name=f"a_nat{_rep}_{it}")
                            nc.gpsimd.dma_start(
                                out=a_nat[:],
                                in_=ann_r[b, tt].rearrange("s p f -> p s f"),
                            )
                        # interleave one GRU weight chunk DMA per iteration
                        if it < len(gru_w_chunks):
                            wsb, wr, fb = gru_w_chunks[it]
                            nc.gpsimd.dma_start(out=wsb[:, fb, :], in_=wr[:, fb, :])

                        # transpose to [f, t]: 4 transposes land in one PSUM
                        # tile, then a single wide copy per f-block
                        a_T = annT_p.tile([128, NFB, TT], FP8 if fp8 else BF16,
                                          tag="annT", name=f"a_T{_rep}_{it}")
                        for fb in range(NFB):
                            tp = ps_tp.tile([128, TT], BF16, tag="tp",
                                            name=f"tp{_rep}_{it}_{fb}")
                            for s in range(NS):
                                nc.tensor.transpose(
                                    tp[:, 128 * s : 128 * (s + 1)],
                                    a_nat[:, s, 128 * fb : 128 * (fb + 1)],
                                    ident[:],
                                )
                            nc.vector.tensor_copy(a_T[:, fb, :], tp[:])

                        if _rep == 0 and it == 0:
                            emit_qT()

                        # pre^T = Ua^T ann^T ; tanh(+q) fused on ScalarE
                        t_T = tanh_p.tile([128, NUB, TT], BF16, tag="tanhT",
                                          name=f"t_T{_rep}_{it}")
                        for ub in range(NUB):
                            pp = ps_pre.tile([128, TT], F32, tag="pre",
                                             name=f"pp{_rep}_{it}_{ub}")
                            if fp8:
                                for q in range(2):
                                    nc.tensor.matmul(
                                        pp[:],
                                        ua8[:, 2 * q : 2 * q + 2,
                                            128 * ub : 128 * (ub + 1)],
                                        a_T[:, 2 * q : 2 * q + 2, :],
                                        start=(q == 0),
                                        stop=(q == 1),
                                        perf_mode=mybir.MatmulPerfMode.DoubleRow,
                                    )
                            else:
                                for fb in range(NFB):
                                    nc.tensor.matmul(
                                        pp[:],
                                        ua_sb[:, fb, 128 * ub : 128 * (ub + 1)],
                                        a_T[:, fb, :],
                                        start=(fb == 0),
                                        stop=(fb == NFB - 1),
                                    )
                            nc.scalar.activation(
                                t_T[:, ub, :], pp[:], AF.Tanh,
                                bias=qT[:, ub, b : b + 1],
                            )

                        # scores (replicated across partitions) + exp + Z part
                        sp = ps_sc.tile([128, TT], F32, tag="sc",
                                        name=f"sp{_rep}_{it}")
                        for ub in range(NUB):
                            nc.tensor.matmul(
                                sp[:],
                                va_rep[:, ub, :],
                                t_T[:, ub, :],
                                start=(ub == 0),
                                stop=(ub == NUB - 1),
                            )
                        p_bc = pbc_p.tile([128, TT], BF16, tag="pbc",
                                          name=f"p_bc{_rep}_{it}")
                        nc.scalar.activation(
                            p_bc[:], sp[:], AF.Exp,
                            accum_out=ztile[:, it : it + 1],
                        )

                        # context partials: cpart[f, fb, tt, b] = sum_t annT*p
                        # multiplies split DVE/GpSimd, reductions split
                        # DVE/ScalarE, so no single engine paces the loop
                        for fb in range(NFB):
                            scr = scr_p.tile([128, TT], BF16, tag="scr",
                                             name=f"scr{_rep}_{it}_{fb}")
                            if fp8 and fb >= 2:
                                nc.gpsimd.tensor_mul(scr[:], a_T[:, fb, :], p_bc[:])
                            else:
                                nc.vector.tensor_mul(scr[:], a_T[:, fb, :], p_bc[:])
                            if fb in (0, 3):
                                nc.vector.reduce_sum(
                                    cpart[:, fb, tt, b : b + 1], scr[:],
                                    axis=mybir.AxisListType.X,
                                )
                            else:
                                scr2 = scr_p.tile([128, TT], BF16, tag="scr2",
                                                  name=f"scr2{_rep}_{it}_{fb}")
                                nc.scalar.activation(
                                    scr2[:], scr[:], AF.Identity,
                                    accum_out=cpart[:, fb, tt, b : b + 1],
                                )

                # ---------------- softmax normalization ----------------
                zsum = state.tile([128, BL], F32, name=f"zsum{_rep}", tag="zsum")
                rz = state.tile([128, BL], F32, name=f"rz{_rep}", tag="rz")
                for b in range(BL):
                    nc.vector.reduce_sum(
                        zsum[:, b : b + 1],
                        ztile[:, b * NTT : (b + 1) * NTT],
                        axis=mybir.AxisListType.X,
                    )
                nc.vector.reciprocal(rz[:], zsum[:])

                cT = state.tile([128, NFB, BL], BF16, name=f"cT{_rep}", tag="cT")
                csum = state.tile([128, NFB, BL], F32, name=f"csum{_rep}", tag="csum")
                for b in range(BL):
                    for fb in range(NFB):
                        nc.vector.reduce_sum(
                            csum[:, fb, b : b + 1],
                            cpart[:, fb, :, b],
                            axis=mybir.AxisListType.X,
                        )
                    nc.vector.tensor_scalar(
                        out=cT[:, :, b],
                        in0=csum[:, :, b],
                        scalar1=rz[:, b : b + 1],
                        scalar2=None,
                        op0=ALU.mult,
                    )

                # ---------------- GRU ----------------
                # context-independent matmuls for all three gate blocks first
                # (they only need x/h and weights, so the PE can chew on them
                # while the DVE finishes the context reduction)
                g_ps = []
                for nb in range(3):
                    if nb % 2 == 0:
                        gp = ps_pre.tile([4, TT], F32, tag="pre", name=f"g_ps{_rep}_{nb}")
                    else:
                        gp = ps_sc.tile([4, TT], F32, tag="sc", name=f"g_ps{_rep}_{nb}")
                    n0 = nb * TT
                    for fb in range(NFB):
                        nc.tensor.matmul(
                            gp[:], xT[:, fb, :], k_sb[:, fb, n0 : n0 + TT],
                            start=(fb == 0), stop=False,
                        )
                    if nb < 2:
                        for ub in range(NUB):
                            nc.tensor.matmul(
                                gp[:], hT[:, ub, :], rk_sb[:, ub, n0 : n0 + TT],
                                start=False, stop=False,
                            )
                    nc.tensor.matmul(
                        gp[:], ones4[:], bias_row[0:1, n0 : n0 + TT],
                        start=False, stop=False,
                    )
                    nc.tensor.matmul(
                        gp[:], ones4[:], abias_row[0:1, n0 : n0 + TT],
                        start=False, stop=False,
                    )
                    g_ps.append(gp)
                for nb in range(3):
                    gp, n0 = g_ps[nb], nb * TT
                    for fb in range(NFB):
                        nc.tensor.matmul(
                            gp[:], cT[:, fb, :], ak_sb[:, fb, n0 : n0 + TT],
                            start=False, stop=(fb == NFB - 1),
                        )

                def hard_sigmoid(dst, src, nm):
                    tmp = state.tile([BL, U], F32, name=f"hs_tmp_{_rep}_{nm}", tag="hs_tmp")
                    nc.vector.tensor_scalar(
                        out=tmp[:], in0=src, scalar1=0.2, scalar2=0.5,
                        op0=ALU.mult, op1=ALU.add,
                    )
                    nc.vector.tensor_scalar(
                        out=dst, in0=tmp[:], scalar1=0.0, scalar2=1.0,
                        op0=ALU.max, op1=ALU.min,
                    )

                z_sb = state.tile([BL, U], F32, name=f"z_sb{_rep}", tag="z_sb")
                r_sb = state.tile([BL, U], F32, name=f"r_sb{_rep}", tag="r_sb")
                hard_sigmoid(z_sb[:], g_ps[0][:], "z")
                hard_sigmoid(r_sb[:], g_ps[1][:], "r")

                rh_bf = state.tile([BL, U], BF16, name=f"rh_bf{_rep}", tag="rh_bf")
                nc.vector.tensor_mul(rh_bf[:], r_sb[:], h_f32[:])
                rhT = state.tile([128, NUB, BL], BF16, name=f"rhT{_rep}", tag="rhT")
                for ub in range(NUB):
                    tp = ps_tp.tile([128, 128], BF16, tag="tp",
                                    name=f"tpg{_rep}_{ub}")
                    nc.tensor.transpose(
                        tp[:, 0:BL], rh_bf[0:BL, 128 * ub : 128 * (ub + 1)],
                        ident[0:BL, 0:BL],
                    )
                    nc.any.tensor_copy(rhT[:, ub, :], tp[:, 0:BL])

                hh_ps = ps_tp.tile([4, TT], F32, tag="tp", name=f"hh_ps{_rep}")
                for ub in range(NUB):
                    nc.tensor.matmul(
                        hh_ps[:], rhT[:, ub, :], rk_sb[:, ub, 2 * U : 3 * U],
                        start=(ub == 0), stop=(ub == NUB - 1),
                    )

                xh_sb = state.tile([BL, U], F32, name=f"xh_sb{_rep}", tag="xh_sb")
                nc.any.tensor_copy(xh_sb[:], g_ps[2][:])
                hh_pre = state.tile([BL, U], F32, name=f"hh_pre{_rep}", tag="hh_pre")
                nc.vector.tensor_add(hh_pre[:], xh_sb[:], hh_ps[:])
                hh = state.tile([BL, U], F32, name=f"hh{_rep}", tag="hh")
                nc.scalar.activation(hh[:], hh_pre[:], AF.Tanh)

                # h_new = hh + z * (h - hh)
                d_sb = state.tile([BL, U], F32, name=f"d_sb{_rep}", tag="d_sb")
                nc.vector.tensor_sub(d_sb[:], h_f32[:], hh[:])
                zd = state.tile([BL, U], F32, name=f"zd{_rep}", tag="zd")
                nc.vector.tensor_mul(zd[:], z_sb[:], d_sb[:])
                out_sb = state.tile([BL, U], F32, name=f"out_sb{_rep}", tag="out_sb")
                nc.vector.tensor_add(out_sb[:], hh[:], zd[:])
                nc.sync.dma_start(out=d_out, in_=out_sb[:])

    nc.compile()
    return nc


_NC = None


def _get_nc():
    global _NC
    if _NC is None:
        _NC = build()
    return _NC


def kernel(**inputs):
    nc = _get_nc()
    shared = {
        k: np.ascontiguousarray(np.asarray(inputs[k], np.float32))
        for k in (
            "kernel", "recurrent_kernel", "attention_kernel", "Wa", "Ua", "Va",
            "bias", "attention_bias", "Wa_bias", "Ua_bias",
        )
    }
    in_maps = []
    for c in range(NCORES):
        sl = slice(c * BL, (c + 1) * BL)
        m = dict(shared)
        m["x"] = np.ascontiguousarray(np.asarray(inputs["x"], np.float32)[sl])
        m["h"] = np.ascontiguousarray(np.asarray(inputs["h"], np.float32)[sl])
        m["annotations"] = np.ascontiguousarray(
            np.asarray(inputs["annotations"], np.float32)[sl]
        )
        in_maps.append(m)
    res = bass_utils.run_bass_kernel_spmd(nc, in_maps, core_ids=list(range(NCORES)))
    return np.concatenate([r["h_new"] for r in res.results], axis=0)



# revision 20
# speedup vs baseline: 1.1874x; 1.1874x over previous
"""Bahdanau-attention GRU cell fused Trainium2 kernel.

Sharding: data-parallel over batch across 8 NeuronCores (4 batch rows per
core, weights replicated, no collectives).

Math per core (b=4 local batch rows, T=2048, F=U=512):
  annotations stream in natural [t, f] layout (DMA casts fp32->bf16),
  PE transposes them to annT [f, t] (bf16), engine copies cast to fp8.
  pre^T[u,t] = Ua^T annT via fp8 DoubleRow matmuls (4x bf16 throughput);
  tanh fused on ScalarE with per-partition bias qT = Wa h + biases,
  output in fp8.
  scores = Va . tanh(pre) via fp8 DoubleRow with Va replicated across
  the 128 output partitions; exp on ScalarE (no max-sub; |scores| <~ 20)
  with accum_out collecting the softmax normalizer Z.
  context: p row is PE-transposed to columns, then c[b] accumulates in
  PSUM via PE matmuls (stationary = p column chunk, moving = natural
  annotation tile) - the whole softmax-weighted sum costs PE only.
  The p-transpose + context matmuls for tile i are emitted during tile
  i+1 so the PE never stalls waiting for ScalarE's exp.
  GRU gates: one PSUM accumulation of x@K + c@AK + h@RK[:,:2U] + biases,
  hard-sigmoid/tanh epilogue, h_new = z*h + (1-z)*hh.
  GRU/attention weights are loaded and cast once (rep 0) and stay
  SBUF-resident across reps; x/h-dependent prep re-runs every rep.
"""

import sys

if "/opt/trn_rl_repo" not in sys.path:
    sys.path.insert(0, "/opt/trn_rl_repo")

import numpy as np

import concourse.bass as bass
import concourse.tile as tile
from concourse import bacc, bass_utils, mybir
from concourse.masks import make_identity

F32 = mybir.dt.float32
BF16 = mybir.dt.bfloat16
FP8 = mybir.dt.float8e4
AF = mybir.ActivationFunctionType
ALU = mybir.AluOpType
DR = mybir.MatmulPerfMode.DoubleRow

B, T, F, U = 32, 2048, 512, 512
NCORES = 8
BL = B // NCORES          # 4 local batch rows
TT = 512                  # T-tile (free dim of matmuls)
NTT = T // TT             # 4
NS = TT // 128            # 4 t-subtiles per T-tile
NFB = F // 128            # 4 f blocks
NUB = U // 128            # 4 u blocks
U3 = 3 * U


def build(reps=1, fp8_scores=True, bufs=None):
    bufs = dict(
        dict(annio=4, aT=4, tanh=3, prow=4, pcol=3, tp=3, pre=3, sc=1, c=1),
        **(bufs or {}),
    )
    nc = bacc.Bacc("TRN2", target_bir_lowering=False, debug=False)

    def din(name, shape):
        return nc.dram_tensor(name, shape, F32, kind="ExternalInput").ap()

    d_x = din("x", [BL, F])
    d_h = din("h", [BL, U])
    d_ann = din("annotations", [BL, T, F])
    d_k = din("kernel", [F, U3])
    d_rk = din("recurrent_kernel", [U, U3])
    d_ak = din("attention_kernel", [F, U3])
    d_wa = din("Wa", [U, U])
    d_ua = din("Ua", [F, U])
    d_va = din("Va", [U])
    d_bias = din("bias", [U3])
    d_abias = din("attention_bias", [U3])
    d_wab = din("Wa_bias", [U])
    d_uab = din("Ua_bias", [U])
    d_out = nc.dram_tensor("h_new", [BL, U], F32, kind="ExternalOutput").ap()

    with tile.TileContext(nc) as tc:
        with (
            tc.tile_pool(name="const", bufs=1) as const,
            tc.tile_pool(name="state", bufs=2) as state,
            tc.tile_pool(name="annio", bufs=bufs["annio"]) as annio,
            tc.tile_pool(name="aT_p", bufs=bufs["aT"]) as aT_p,
            tc.tile_pool(name="tanh_p", bufs=bufs["tanh"]) as tanh_p,
            tc.tile_pool(name="prow_p", bufs=bufs["prow"]) as prow_p,
            tc.tile_pool(name="pcol_p", bufs=bufs["pcol"]) as pcol_p,
            tc.tile_pool(name="ps_tp", bufs=bufs["tp"], space="PSUM") as ps_tp,
            tc.tile_pool(name="ps_pre", bufs=bufs["pre"], space="PSUM") as ps_pre,
            tc.tile_pool(name="ps_sc", bufs=bufs["sc"], space="PSUM") as ps_sc,
            tc.tile_pool(name="ps_c", bufs=bufs["c"], space="PSUM") as ps_c,
        ):
            # ---------------- constants / weights ----------------
            ident = const.tile([128, 128], BF16)
            make_identity(nc, ident[:])

            ones4 = const.tile([1, BL], BF16)
            nc.vector.memset(ones4[:], 1.0)
            ident_f1 = const.tile([1, 1], F32)
            nc.vector.memset(ident_f1[:], 1.0)

            # first annotation tile: issue before any weight DMA so the PE
            # pipeline can start transposing as early as possible
            ann_r = d_ann.rearrange("b (tt s p) f -> b tt s p f", p=128, s=NS)
            a_nat0 = annio.tile([128, NS, F], BF16, tag="ann_nat", name="a_nat_first")
            nc.gpsimd.dma_start(
                out=a_nat0[:], in_=ann_r[0, 0].rearrange("s p f -> p s f")
            )

            ua_sb = const.tile([128, NFB, U], BF16)
            nc.gpsimd.dma_start(
                out=ua_sb[:], in_=d_ua.rearrange("(fb p) u -> p fb u", p=128)
            )
            ua8 = const.tile([128, NFB, U], FP8)
            nc.vector.tensor_copy(ua8[:], ua_sb[:])
            wa_sb = const.tile([128, NUB, U], BF16)
            nc.gpsimd.dma_start(
                out=wa_sb[:], in_=d_wa.rearrange("(jb p) u -> p jb u", p=128)
            )

            # small vectors: fast HWDGE fp32 loads + on-chip casts
            def row_load(dram_ap, width, nm):
                t32 = const.tile([1, width], F32, name=nm + "_f32")
                nc.sync.dma_start(out=t32[:], in_=dram_ap)
                t16 = const.tile([1, width], BF16, name=nm)
                nc.vector.tensor_copy(t16[:], t32[:])
                return t16

            va_row = row_load(d_va.rearrange("(a u) -> a u", a=1), U, "va_row")
            wab_row = row_load(d_wab.rearrange("(a u) -> a u", a=1), U, "wab_row")
            uab_row = row_load(d_uab.rearrange("(a u) -> a u", a=1), U, "uab_row")
            bias_row = row_load(d_bias.rearrange("(a u) -> a u", a=1), U3, "bias_row")
            abias_row = row_load(d_abias.rearrange("(a u) -> a u", a=1), U3, "abias_row")

            # GRU weights: loaded once (rep 0), SBUF-resident afterwards
            k_sb = const.tile([128, NFB, U3], BF16)
            rk_sb = const.tile([128, NUB, U3], BF16)
            ak_sb = const.tile([128, NFB, U3], BF16)
            k_r = d_k.rearrange("(fb p) u -> p fb u", p=128)
            rk_r = d_rk.rearrange("(fb p) u -> p fb u", p=128)
            ak_r = d_ak.rearrange("(fb p) u -> p fb u", p=128)
            gru_w_chunks = []
            for fb in range(NFB):
                gru_w_chunks.append((k_sb, k_r, fb))
                gru_w_chunks.append((rk_sb, rk_r, fb))
                gru_w_chunks.append((ak_sb, ak_r, fb))

            # VaT replicated: va_rep[p, ub, j] = Va[ub*128+p] for all j
            va_rep = const.tile([128, NUB, 128], BF16)
            for ub in range(NUB):
                tp = ps_tp.tile([128, 128], BF16, tag="tp")
                nc.tensor.transpose(
                    tp[:, 0:1], va_row[0:1, 128 * ub : 128 * (ub + 1)], ident[0:1, 0:1]
                )
                nc.vector.tensor_copy(
                    va_rep[:, ub, :], tp[:, 0:1].to_broadcast([128, 128])
                )
            if fp8_scores:
                va8 = const.tile([128, NUB, 128], FP8)
                nc.vector.tensor_copy(va8[:], va_rep[:])

            for _rep in range(reps):
                # ---- per-rep x/h-dependent prep (emitted inside tile 0 so
                # the PE starts on annotation transposes first) ----
                x_f32 = state.tile([BL, F], F32, tag="x_f32", name=f"x_f32_{_rep}")
                h_f32 = state.tile([BL, U], F32, tag="h_f32", name=f"h_f32_{_rep}")
                x_bf = state.tile([BL, F], BF16, tag="x_bf", name=f"x_bf_{_rep}")
                h_bf = state.tile([BL, U], BF16, tag="h_bf", name=f"h_bf_{_rep}")
                xT = state.tile([128, NFB, BL], BF16, tag="xT", name=f"xT_{_rep}")
                hT = state.tile([128, NUB, BL], BF16, tag="hT", name=f"hT_{_rep}")
                qT = state.tile([128, NUB, BL], F32, tag="qT", name=f"qT_{_rep}")

                def emit_xhq(r=_rep, x_f32=x_f32, h_f32=h_f32, x_bf=x_bf,
                             h_bf=h_bf, xT=xT, hT=hT, qT=qT):
                    nc.sync.dma_start(out=x_f32[:], in_=d_x)
                    nc.sync.dma_start(out=h_f32[:], in_=d_h)
                    nc.vector.tensor_copy(x_bf[:], x_f32[:])
                    nc.vector.tensor_copy(h_bf[:], h_f32[:])
                    for jb in range(NFB):
                        tp = ps_tp.tile([128, 128], BF16, tag="tp",
                                        name=f"tpx{r}_{jb}")
                        nc.tensor.transpose(
                            tp[:, 0:BL], x_bf[0:BL, 128 * jb : 128 * (jb + 1)],
                            ident[0:BL, 0:BL],
                        )
                        nc.any.tensor_copy(xT[:, jb, :], tp[:, 0:BL])
                    for jb in range(NUB):
                        tp = ps_tp.tile([128, 128], BF16, tag="tp",
                                        name=f"tph{r}_{jb}")
                        nc.tensor.transpose(
                            tp[:, 0:BL], h_bf[0:BL, 128 * jb : 128 * (jb + 1)],
                            ident[0:BL, 0:BL],
                        )
                        nc.any.tensor_copy(hT[:, jb, :], tp[:, 0:BL])
                    # qT[u, b] = Wa^T h^T + Wa_bias + Ua_bias
                    for ub in range(NUB):
                        qp = ps_sc.tile([128, TT], F32, tag="sc", name=f"qp{r}_{ub}")
                        for jb in range(NUB):
                            nc.tensor.matmul(
                                qp[:, 0:BL],
                                wa_sb[:, jb, 128 * ub : 128 * (ub + 1)],
                                hT[:, jb, :],
                                start=(jb == 0),
                                stop=False,
                            )
                        nc.tensor.matmul(
                            qp[:, 0:BL],
                            wab_row[0:1, 128 * ub : 128 * (ub + 1)],
                            ones4[:],
                            start=False,
                            stop=False,
                        )
                        nc.tensor.matmul(
                            qp[:, 0:BL],
                            uab_row[0:1, 128 * ub : 128 * (ub + 1)],
                            ones4[:],
                            start=False,
                            stop=True,
                        )
                        nc.any.tensor_copy(qT[:, ub, :], qp[:, 0:BL])

                # per-rep accumulators
                ztile = state.tile([1, BL * NTT], F32, name=f"ztile{_rep}",
                                   tag="ztile")
                c_ps = ps_c.tile([BL, F], F32, name=f"c_ps{_rep}", tag="c_ps")

                # context-independent GRU gate parts (x@K + h@RK + biases),
                # computed mid-loop and parked in SBUF; re-injected into the
                # gate PSUM accumulation via an identity stationary in the tail
                xh_pre = state.tile([BL, 3, U], BF16, name=f"xh_pre{_rep}",
                                    tag="xh_pre")

                def emit_gate_pre(nb, r=_rep, xh_pre=xh_pre):
                    gp = ps_pre.tile([4, TT], F32, tag="pre", name=f"gpre{r}_{nb}")
                    n0 = nb * TT
                    for fb in range(NFB):
                        nc.tensor.matmul(
                            gp[:], xT[:, fb, :], k_sb[:, fb, n0 : n0 + TT],
                            start=(fb == 0), stop=False,
                        )
                    if nb < 2:
                        for ub in range(NUB):
                            nc.tensor.matmul(
                                gp[:], hT[:, ub, :], rk_sb[:, ub, n0 : n0 + TT],
                                start=False, stop=False,
                            )
                    nc.tensor.matmul(
                        gp[:], ones4[:], bias_row[0:1, n0 : n0 + TT],
                        start=False, stop=False,
                    )
                    nc.tensor.matmul(
                        gp[:], ones4[:], abias_row[0:1, n0 : n0 + TT],
                        start=False, stop=True,
                    )
                    nc.vector.tensor_copy(xh_pre[:, nb, :], gp[:])

                # deferred context emission: p-transpose + context matmuls of
                # tile i run during tile i+1 so the PE never waits on exp(i).
                # The stationary is a [128, BL] tile with the p column in
                # slot b and zeros elsewhere, so the [BL, F] PSUM region
                # accumulates row b only (matmul out base partition must be 0).
                def emit_ctx(b, tt, p_row, a_nat, r=_rep, c_ps=c_ps):
                    it = b * NTT + tt
                    p_colT = ps_tp.tile([128, NS], F32, tag="tp",
                                        name=f"pcT{r}_{b}_{tt}")
                    for s in range(NS):
                        nc.tensor.transpose(
                            p_colT[:, s : s + 1],
                            p_row[0:1, 128 * s : 128 * (s + 1)],
                            ident_f1[:],
                        )
                    p_col = pcol_p.tile([128, NS, BL], BF16, tag="pcol",
                                        name=f"pc{r}_{b}_{tt}")
                    nc.gpsimd.memset(p_col[:], 0.0)
                    nc.vector.tensor_copy(p_col[:, :, b], p_colT[:])
                    for s in range(NS):
                        nc.tensor.matmul(
                            c_ps[:],
                            p_col[:, s, :],
                            a_nat[:, s, :],
                            start=(it == 0 and s == 0),
                            stop=(it == BL * NTT - 1 and s == NS - 1),
                        )

                pend = None

                # ---------------- main attention loop ----------------
                for b in range(BL):
                    for tt in range(NTT):
                        it = b * NTT + tt
                        if _rep == 0 and it == 0:
                            a_nat = a_nat0
                        else:
                            a_nat = annio.tile([128, NS, F], BF16, tag="ann_nat",
                                               name=f"a_nat{_rep}_{it}")
                            nc.gpsimd.dma_start(
                                out=a_nat[:],
                                in_=ann_r[b, tt].rearrange("s p f -> p s f"),
                            )
                        # GRU weight loads interleaved into rep 0 only
                        if _rep == 0 and it < len(gru_w_chunks):
                            wsb, wr, fb = gru_w_chunks[it]
                            nc.gpsimd.dma_start(out=wsb[:, fb, :], in_=wr[:, fb, :])

                        # transpose to [f, t]; psum->sbuf copies cast to fp8,
                        # split across DVE and Pool
                        a_T = aT_p.tile([128, NFB, TT], FP8, tag="annT",
                                        name=f"a_T{_rep}_{it}")
                        for fb in range(NFB):
                            tp = ps_tp.tile([128, TT], BF16, tag="tp",
                                            name=f"tp{_rep}_{it}_{fb}")
                            for s in range(NS):
                                nc.tensor.transpose(
                                    tp[:, 128 * s : 128 * (s + 1)],
                                    a_nat[:, s, 128 * fb : 128 * (fb + 1)],
                                    ident[:],
                                )
                            nc.vector.tensor_copy(a_T[:, fb, :], tp[:])

                        if it == 0:
                            emit_xhq()

                        # pre^T = Ua^T annT in fp8 DoubleRow; tanh on ScalarE
                        t_T = tanh_p.tile([128, NUB, TT],
                                          FP8 if fp8_scores else BF16,
                                          tag="tanhT", name=f"t_T{_rep}_{it}")
                        for ub in range(NUB):
                            pp = ps_pre.tile([128, TT], F32, tag="pre",
                                             name=f"pp{_rep}_{it}_{ub}")
                            for q in range(2):
                                nc.tensor.matmul(
                                    pp[:],
                                    ua8[:, 2 * q : 2 * q + 2,
                                        128 * ub : 128 * (ub + 1)],
                                    a_T[:, 2 * q : 2 * q + 2, :],
                                    start=(q == 0),
                                    stop=(q == 1),
                                    perf_mode=DR,
                                )
                            nc.scalar.activation(
                                t_T[:, ub, :], pp[:], AF.Tanh,
                                bias=qT[:, ub, b : b + 1],
                            )

                        # scores (replicated across partitions)
                        sp = ps_sc.tile([128, TT], F32, tag="sc",
                                        name=f"sp{_rep}_{it}")
                        if fp8_scores:
                            for m in range(2):
                                nc.tensor.matmul(
                                    sp[:],
                                    va8[:, 2 * m : 2 * m + 2, :],
                                    t_T[:, 2 * m : 2 * m + 2, :],
                                    start=(m == 0),
                                    stop=(m == 1),
                                    perf_mode=DR,
                                )
                        else:
                            for ub in range(NUB):
                                nc.tensor.matmul(
                                    sp[:],
                                    va_rep[:, ub, :],
                                    t_T[:, ub, :],
                                    start=(ub == 0),
                                    stop=(ub == NUB - 1),
                                )

                        # deferred context for the previous tile
                        if pend is not None:
                            emit_ctx(*pend)

                        # exp + Z partial (row 0 only; no max-sub needed)
                        p_row = prow_p.tile([1, TT], F32, tag="prow",
                                            name=f"p_row{_rep}_{it}")
                        nc.scalar.activation(
                            p_row[:], sp[0:1, :], AF.Exp,
                            accum_out=ztile[:, it : it + 1],
                        )
                        pend = (b, tt, p_row, a_nat)

                        if it in (12, 13, 14):
                            emit_gate_pre(it - 12)

                emit_ctx(*pend)

                # ---------------- softmax normalization ----------------
                zsum = state.tile([1, BL], F32, name=f"zsum{_rep}", tag="zsum")
                rz = state.tile([1, BL], F32, name=f"rz{_rep}", tag="rz")
                for b in range(BL):
                    nc.vector.reduce_sum(
                        zsum[0:1, b : b + 1],
                        ztile[0:1, b * NTT : (b + 1) * NTT],
                        axis=mybir.AxisListType.X,
                    )
                nc.vector.reciprocal(rz[:], zsum[:])
                rzT_ps = ps_tp.tile([BL, 1], F32, tag="tp", name=f"rzT{_rep}")
                nc.tensor.transpose(rzT_ps[:], rz[0:1, :], ident_f1[:])
                rz4 = state.tile([BL, 1], F32, name=f"rz4{_rep}", tag="rz4")
                nc.any.tensor_copy(rz4[:], rzT_ps[:])

                # c rows normalized + cast, then transposed for the GRU
                c_rows = state.tile([BL, F], BF16, name=f"c_rows{_rep}",
                                    tag="c_rows")
                nc.vector.tensor_scalar(
                    out=c_rows[:], in0=c_ps[:], scalar1=rz4[:, 0:1],
                    scalar2=None, op0=ALU.mult,
                )
                cT = state.tile([128, NFB, BL], BF16, name=f"cT{_rep}", tag="cT")
                for fb in range(NFB):
                    tp = ps_tp.tile([128, 128], BF16, tag="tp",
                                    name=f"tpc{_rep}_{fb}")
                    nc.tensor.transpose(
                        tp[:, 0:BL], c_rows[0:BL, 128 * fb : 128 * (fb + 1)],
                        ident[0:BL, 0:BL],
                    )
                    nc.any.tensor_copy(cT[:, fb, :], tp[:, 0:BL])

                # ---------------- GRU ----------------
                # re-inject the precomputed gate parts (identity stationary),
                # then accumulate the context contribution c @ AK
                g_ps = []
                for nb in range(3):
                    if nb % 2 == 0:
                        gp = ps_pre.tile([4, TT], F32, tag="pre",
                                         name=f"g_ps{_rep}_{nb}")
                    else:
                        gp = ps_sc.tile([4, TT], F32, tag="sc",
                                        name=f"g_ps{_rep}_{nb}")
                    n0 = nb * TT
                    nc.tensor.matmul(
                        gp[:], ident[0:BL, 0:BL], xh_pre[:, nb, :],
                        start=True, stop=False,
                    )
                    for fb in range(NFB):
                        nc.tensor.matmul(
                            gp[:], cT[:, fb, :], ak_sb[:, fb, n0 : n0 + TT],
                            start=False, stop=(fb == NFB - 1),
                        )
                    g_ps.append(gp)

                def hard_sigmoid(dst, src, nm):
                    tmp = state.tile([BL, U], F32, name=f"hs_tmp_{_rep}_{nm}",
                                     tag="hs_tmp")
                    nc.vector.tensor_scalar(
                        out=tmp[:], in0=src, scalar1=0.2, scalar2=0.5,
                        op0=ALU.mult, op1=ALU.add,
                    )
                    nc.vector.tensor_scalar(
                        out=dst, in0=tmp[:], scalar1=0.0, scalar2=1.0,
                        op0=ALU.max, op1=ALU.min,
                    )

                z_sb = state.tile([BL, U], F32, name=f"z_sb{_rep}", tag="z_sb")
                r_sb = state.tile([BL, U], F32, name=f"r_sb{_rep}", tag="r_sb")
                hard_sigmoid(z_sb[:], g_ps[0][:], "z")
                hard_sigmoid(r_sb[:], g_ps[1][:], "r")

                rh_bf = state.tile([BL, U], BF16, name=f"rh_bf{_rep}", tag="rh_bf")
                nc.vector.tensor_mul(rh_bf[:], r_sb[:], h_f32[:])
                rhT = state.tile([128, NUB, BL], BF16, name=f"rhT{_rep}", tag="rhT")
                for ub in range(NUB):
                    tp = ps_tp.tile([128, 128], BF16, tag="tp",
                                    name=f"tpg{_rep}_{ub}")
                    nc.tensor.transpose(
                        tp[:, 0:BL], rh_bf[0:BL, 128 * ub : 128 * (ub + 1)],
                        ident[0:BL, 0:BL],
                    )
                    nc.any.tensor_copy(rhT[:, ub, :], tp[:, 0:BL])

                hh_ps = ps_pre.tile([4, TT], F32, tag="pre", name=f"hh_ps{_rep}")
                for ub in range(NUB):
                    nc.tensor.matmul(
                        hh_ps[:], rhT[:, ub, :], rk_sb[:, ub, 2 * U : 3 * U],
                        start=(ub == 0), stop=(ub == NUB - 1),
                    )

                xh_sb = state.tile([BL, U], F32, name=f"xh_sb{_rep}", tag="xh_sb")
                nc.any.tensor_copy(xh_sb[:], g_ps[2][:])
                hh_pre = state.tile([BL, U], F32, name=f"hh_pre{_rep}", tag="hh_pre")
                nc.vector.tensor_add(hh_pre[:], xh_sb[:], hh_ps[:])
                hh = state.tile([BL, U], F32, name=f"hh{_rep}", tag="hh")
                nc.scalar.activation(hh[:], hh_pre[:], AF.Tanh)

                # h_new = hh + z * (h - hh)
                d_sb = state.tile([BL, U], F32, name=f"d_sb{_rep}", tag="d_sb")
                nc.vector.tensor_sub(d_sb[:], h_f32[:], hh[:])
                zd = state.tile([BL, U], F32, name=f"zd{_rep}", tag="zd")
                nc.vector.tensor_mul(zd[:], z_sb[:], d_sb[:])
                out_sb = state.tile([BL, U], F32, name=f"out_sb{_rep}", tag="out_sb")
                nc.vector.tensor_add(out_sb[:], hh[:], zd[:])
                nc.sync.dma_start(out=d_out, in_=out_sb[:])

    nc.compile()
    return nc


_NC = None


def _get_nc():
    global _NC
    if _NC is None:
        _NC = build()
    return _NC


def kernel(**inputs):
    nc = _get_nc()
    shared = {
        k: np.ascontiguousarray(np.asarray(inputs[k], np.float32))
        for k in (
            "kernel", "recurrent_kernel", "attention_kernel", "Wa", "Ua", "Va",
            "bias", "attention_bias", "Wa_bias", "Ua_bias",
        )
    }
    in_maps = []
    for c in range(NCORES):
        sl = slice(c * BL, (c + 1) * BL)
        m = dict(shared)
        m["x"] = np.ascontiguousarray(np.asarray(inputs["x"], np.float32)[sl])
        m["h"] = np.ascontiguousarray(np.asarray(inputs["h"], np.float32)[sl])
        m["annotations"] = np.ascontiguousarray(
            np.asarray(inputs["annotations"], np.float32)[sl]
        )
        in_maps.append(m)
    res = bass_utils.run_bass_kernel_spmd(nc, in_maps, core_ids=list(range(NCORES)))
    return np.concatenate([r["h_new"] for r in res.results], axis=0)
